# revision 7
# baseline (speedup 1.0000x reference)
"""Causal multi-head attention (B=4, T=2048, D=2048, H=16) on 8 Trainium2
NeuronCores via Bass/Tile, SPMD with zero collectives.

Sharding: each batch b is split over two cores by query rows using a
zigzag quarter split (core 2b: quarters Q1+Q4, core 2b+1: quarters Q2+Q3),
which balances the causal-attention triangle. Every core computes K/V
projections for its batch's full sequence (cheap redundancy that keeps the
SPMD program uniform across cores); causal masking is supplied as per-core
input data over a uniform tile pattern, so all 8 cores run the same
instruction stream.

Per-core pipeline (all matmuls in float32r — full PE rate, ~1e-4 rel err):
  0. PE-transpose x -> xT (SBUF slabs, one T/2 half at a time)
  1. K^T = Wk^T xT, Q^T = Wq^T xT (transposed layouts), V = x Wv (natural)
  2. per head: S^T tiles = K^T_chunk^T Q^T, exp on ACT (no max subtraction:
     scores are O(1) by construction), causal/pad masking by DVE multiply,
     A^T accumulated on PE with V as stationary operand, softmax denominators
     via ones-vector matmuls, normalization fused into the PSUM evacuation
  3. O rows = A^T^T Wo + bo
Outputs are the core's own (permuted) query rows; the host scatters them
back into the full [B, T, D] tensor.
"""
import numpy as np

import concourse.bacc as bacc
import concourse.mybir as mybir
from concourse.tile import TileContext
from concourse.bass_utils import run_bass_kernel_spmd

F32 = mybir.dt.float32
F32R = mybir.dt.float32r
EXP = mybir.ActivationFunctionType.Exp
MULT = mybir.AluOpType.mult

PROD_CFG = dict(B=4, T=2048, D=2048, H=16)


def _derived(cfg):
    B, T, D, H = cfg["B"], cfg["T"], cfg["D"], cfg["H"]
    d = dict(cfg)
    d.update(
        QW=T // 4,            # quarter width (query-row shard unit)
        OWN=T // 2,           # own query rows per core
        T2=T // 2,            # xT slab half width
        DK=D // 128,          # contraction chunks
        q=T // 4 // 128,      # 128-row j-tiles per quarter
        NCH=min(512, T // 2),  # moving-N chunk for projections
        ND=min(1024, D),      # phase-4 output-column half
        DH=128,
        N_CORES=2 * B,
    )
    return d


def _r(ap):
    return ap.bitcast(F32R)


def build_nc(cfg):
    c = _derived(cfg)
    T, D, H = c["T"], c["D"], c["H"]
    QW, OWN, T2, DK, q = c["QW"], c["OWN"], c["T2"], c["DK"], c["q"]
    NCH, ND = c["NCH"], c["ND"]
    SCALE = float(c["DH"] ** -0.5)

    nc = bacc.Bacc(
        "TRN2", target_bir_lowering=False, debug=False, num_devices=c["N_CORES"]
    )
    x = nc.dram_tensor("x", [T, D], F32R, kind="ExternalInput").ap()
    wq = nc.dram_tensor("wq", [D, D], F32R, kind="ExternalInput").ap()
    wk = nc.dram_tensor("wk", [D, D], F32R, kind="ExternalInput").ap()
    wv = nc.dram_tensor("wv", [D, D], F32R, kind="ExternalInput").ap()
    wo = nc.dram_tensor("wo", [D, D], F32R, kind="ExternalInput").ap()
    bq = nc.dram_tensor("bq", [D], F32, kind="ExternalInput").ap()
    bk = nc.dram_tensor("bk", [D], F32, kind="ExternalInput").ap()
    bv = nc.dram_tensor("bv", [D], F32, kind="ExternalInput").ap()
    bo = nc.dram_tensor("bo", [D], F32, kind="ExternalInput").ap()
    mask = nc.dram_tensor("mask", [128, 4 * q * QW], F32R, kind="ExternalInput").ap()
    ident_in = nc.dram_tensor("ident", [128, 128], F32R, kind="ExternalInput").ap()
    ones_c_in = nc.dram_tensor("ones_c", [128, 1], F32R, kind="ExternalInput").ap()
    ones_r_in = nc.dram_tensor("ones_r", [1, 128], F32R, kind="ExternalInput").ap()
    o = nc.dram_tensor("o", [OWN, D], F32, kind="ExternalOutput").ap()

    kt_d = nc.dram_tensor("kt_scratch", [D, T], F32R).ap()
    qt_d = nc.dram_tensor("qt_scratch", [D, OWN], F32R).ap()
    v_d = nc.dram_tensor("v_scratch", [T, D], F32R).ap()
    at_d = nc.dram_tensor("at_scratch", [D, OWN], F32R).ap()
    l_d = nc.dram_tensor("l_scratch", [H, OWN], F32).ap()

    # uniform causal j-tile windows (see module docstring)
    LWIN = list(range(q)) + list(range(2 * q, 3 * q))          # L+H valid
    HONLY = list(range(q, 2 * q)) + list(range(3 * q, 4 * q))  # H valid only

    with TileContext(nc) as tc:
        with (
            tc.tile_pool(name="const", bufs=1) as pconst,
            tc.tile_pool(name="slab", bufs=1) as pslab,
        ):
            ident = pconst.tile([128, 128], F32R, tag="ident")
            nc.sync.dma_start(out=ident[:], in_=ident_in[:])
            ones_col = pconst.tile([128, 1], F32R, tag="ones_col")
            nc.sync.dma_start(out=ones_col[:], in_=ones_c_in[:])
            ones_row = pconst.tile([1, 128], F32R, tag="ones_row")
            nc.sync.dma_start(out=ones_row[:], in_=ones_r_in[:])
            bk_sb = pconst.tile([128, DK], F32, tag="bk")
            nc.sync.dma_start(out=bk_sb[:], in_=bk.rearrange("(m p) -> p m", p=128))
            bq_sb = pconst.tile([128, DK], F32, tag="bq")
            nc.sync.dma_start(out=bq_sb[:], in_=bq.rearrange("(m p) -> p m", p=128))
            bv_sb = pconst.tile([1, D], F32R, tag="bv")
            nc.sync.dma_start(out=bv_sb[:], in_=bv[None, :].bitcast(F32R))
            bo_sb = pconst.tile([1, D], F32R, tag="bo")
            nc.sync.dma_start(out=bo_sb[:], in_=bo[None, :].bitcast(F32R))

            # ---------------- phase 0+1: xT, K^T, Q^T, V ----------------
            with (
                tc.tile_pool(name="p1x", bufs=2) as p1x,
                tc.tile_pool(name="p1w", bufs=2) as p1w,
                tc.tile_pool(name="p1wv", bufs=2) as p1wv,
                tc.tile_pool(name="p1st", bufs=3) as p1st,
                tc.tile_pool(name="ps_tr", bufs=2, space="PSUM") as ps_tr,
                tc.tile_pool(name="ps_kq", bufs=2, space="PSUM") as ps_kq,
                tc.tile_pool(name="ps_v", bufs=2, space="PSUM") as ps_v,
            ):
                for hf in range(2):
                    slab = pslab.tile([128, DK * T2], F32R, tag="slab")
                    slab3 = slab[:].rearrange("p (k t) -> p k t", k=DK)
                    # transpose x rows [hf*T2, (hf+1)*T2) into slab
                    for tcn in range(T2 // 128):
                        xst = p1x.tile([128, D], F32R, tag="xst")
                        nc.sync.dma_start(
                            out=xst[:],
                            in_=x[hf * T2 + tcn * 128: hf * T2 + (tcn + 1) * 128, :],
                        )
                        for kb in range(0, DK, 4):
                            nb = min(4, DK - kb)
                            ps = ps_tr.tile([128, 512], F32R, tag="pstr")
                            for i in range(nb):
                                nc.tensor.transpose(
                                    ps[:, i * 128:(i + 1) * 128],
                                    xst[:, (kb + i) * 128:(kb + i + 1) * 128],
                                    ident[:],
                                )
                            nc.vector.tensor_copy(
                                slab3[:, kb:kb + nb, tcn * 128:(tcn + 1) * 128],
                                ps[:, : nb * 128].rearrange(
                                    "p (a b) -> p a b", a=nb
                                ),
                            )
                    # K^T (and Q^T on half 0) projections
                    projs = [(wk, bk_sb, kt_d, True)]
                    if hf == 0:
                        projs.append((wq, bq_sb, qt_d, False))
                    for w_in, b_sb, out_d, is_k in projs:
                        for m in range(DK):
                            wm = p1w.tile([128, DK * 128], F32R, tag="wm")
                            nc.sync.dma_start(
                                out=wm[:],
                                in_=w_in.rearrange("(k p) n -> p k n", p=128)[
                                    :, :, m * 128:(m + 1) * 128
                                ],
                            )
                            for jt in range(T2 // NCH):
                                ps = ps_kq.tile([128, NCH], F32, tag="pskq")
                                for k in range(DK):
                                    nc.tensor.matmul(
                                        ps[:],
                                        _r(wm[:, k * 128:(k + 1) * 128]),
                                        _r(slab[:, k * T2 + jt * NCH:
                                                k * T2 + (jt + 1) * NCH]),
                                        start=(k == 0),
                                        stop=(k == DK - 1),
                                    )
                                st = p1st.tile([128, NCH], F32R, tag="kqst")
                                nc.vector.tensor_scalar_add(
                                    st[:], ps[:], b_sb[:, m:m + 1]
                                )
                                col0 = (hf * T2 if is_k else 0) + jt * NCH
                                nc.sync.dma_start(
                                    out=out_d[m * 128:(m + 1) * 128,
                                              col0:col0 + NCH],
                                    in_=st[:],
                                )
                    # V projection (natural layout), n-chunks of 512
                    for nb_ in range(D // min(512, D)):
                        NV = min(512, D)
                        wvn = p1wv.tile([128, DK * NV], F32R, tag="wvn")
                        nc.sync.dma_start(
                            out=wvn[:],
                            in_=wv.rearrange("(k p) n -> p k n", p=128)[
                                :, :, nb_ * NV:(nb_ + 1) * NV
                            ],
                        )
                        for tcn in range(T2 // 128):
                            ps = ps_v.tile([128, NV], F32, tag="psv")
                            for k in range(DK):
                                nc.tensor.matmul(
                                    ps[:],
                                    _r(slab[:, k * T2 + tcn * 128:
                                            k * T2 + (tcn + 1) * 128]),
                                    _r(wvn[:, k * NV:(k + 1) * NV]),
                                    start=(k == 0),
                                    stop=False,
                                )
                            nc.tensor.matmul(
                                ps[:],
                                _r(ones_row[:]),
                                _r(bv_sb[:, nb_ * NV:(nb_ + 1) * NV]),
                                start=False,
                                stop=True,
                            )
                            st = p1st.tile([128, NV], F32R, tag="vst")
                            nc.scalar.copy(st[:], ps[:])
                            nc.sync.dma_start(
                                out=v_d[hf * T2 + tcn * 128:
                                        hf * T2 + (tcn + 1) * 128,
                                        nb_ * NV:(nb_ + 1) * NV],
                                in_=st[:],
                            )

            # ---------------- phase 2+3: attention per head ----------------
            with (
                tc.tile_pool(name="pmask", bufs=1) as pmask,
                tc.tile_pool(name="ph", bufs=2) as ph,
                tc.tile_pool(name="ppt", bufs=3) as ppt,
                tc.tile_pool(name="psm", bufs=2) as psm,
                tc.tile_pool(name="ps_s", bufs=2, space="PSUM") as ps_s,
                tc.tile_pool(name="ps_a", bufs=1, space="PSUM") as ps_a,
                tc.tile_pool(name="ps_l", bufs=1, space="PSUM") as ps_l,
            ):
                mask_sb = pmask.tile([128, 4 * q * QW], F32R, tag="mask")
                nc.sync.dma_start(out=mask_sb[:], in_=mask[:])
                NS = min(512, OWN)
                for h in range(H):
                    kt_h = ph.tile([128, T], F32R, tag="kth")
                    nc.sync.dma_start(
                        out=kt_h[:], in_=kt_d[h * 128:(h + 1) * 128, :]
                    )
                    qt_h = ph.tile([128, OWN], F32R, tag="qth")
                    nc.sync.dma_start(
                        out=qt_h[:], in_=qt_d[h * 128:(h + 1) * 128, :]
                    )
                    v_h = ph.tile([128, T], F32R, tag="vh")
                    nc.sync.dma_start(
                        out=v_h[:].rearrange("p (jb c) -> p jb c", c=128),
                        in_=v_d.rearrange("(jb p) d -> p jb d", p=128)[
                            :, :, h * 128:(h + 1) * 128
                        ],
                    )
                    psa = ps_a.tile([128, OWN], F32, tag="psa")
                    psl = ps_l.tile([1, OWN], F32, tag="psl")
                    order = LWIN + HONLY
                    for idx, jb in enumerate(order):
                        full = idx < len(LWIN)
                        width = OWN if full else QW
                        pss = ps_s.tile([128, OWN], F32, tag="pss")
                        qoff = 0 if full else QW
                        ns = min(NS, width)
                        for sc in range(width // ns):
                            nc.tensor.matmul(
                                pss[:, sc * ns:(sc + 1) * ns],
                                _r(kt_h[:, jb * 128:(jb + 1) * 128]),
                                _r(qt_h[:, qoff + sc * ns: qoff + (sc + 1) * ns]),
                                start=True,
                                stop=True,
                            )
                        pt = ppt.tile([128, OWN], F32R, tag="pt")
                        nc.scalar.activation(
                            pt[:, :width], pss[:, :width], EXP, scale=SCALE
                        )
                        # causal / padding masks (uniform pattern, per-core data)
                        if full:
                            mc = LWIN.index(jb) * QW
                            nc.vector.tensor_mul(
                                pt[:, :QW], pt[:, :QW], mask_sb[:, mc:mc + QW]
                            )
                        else:
                            mc = (2 * q + HONLY.index(jb)) * QW
                            nc.vector.tensor_mul(
                                pt[:, :QW], pt[:, :QW], mask_sb[:, mc:mc + QW]
                            )
                        vt = _r(v_h[:, jb * 128:(jb + 1) * 128])
                        first = idx == 0
                        # start=True clears has_written for the WHOLE bank, so
                        # the H region may only issue a clearing start when it
                        # occupies its own bank; otherwise its first write
                        # relies on overwrite-where-unset after the L clear.
                        h_own_bank = QW * 4 >= 2048
                        first_h = first and h_own_bank
                        last_l = idx == len(LWIN) - 1
                        last_h = idx == len(order) - 1
                        if full:
                            nc.tensor.matmul(
                                psa[:, :QW], vt, _r(pt[:, :QW]),
                                start=first, stop=last_l,
                            )
                            nc.tensor.matmul(
                                psa[:, QW:OWN], vt, _r(pt[:, QW:OWN]),
                                start=first_h, stop=last_h,
                            )
                            nc.tensor.matmul(
                                psl[:, :QW], _r(ones_col[:]), _r(pt[:, :QW]),
                                start=first, stop=last_l,
                            )
                            nc.tensor.matmul(
                                psl[:, QW:OWN], _r(ones_col[:]),
                                _r(pt[:, QW:OWN]),
                                start=first_h, stop=last_h,
                            )
                        else:
                            nc.tensor.matmul(
                                psa[:, QW:OWN], vt, _r(pt[:, :QW]),
                                start=False, stop=last_h,
                            )
                            nc.tensor.matmul(
                                psl[:, QW:OWN], _r(ones_col[:]),
                                _r(pt[:, :QW]),
                                start=False, stop=last_h,
                            )
                    l_sb = psm.tile([1, OWN], F32, tag="lsb")
                    nc.vector.reciprocal(l_sb[:], psl[:])
                    nc.sync.dma_start(out=l_d[h:h + 1, :], in_=l_sb[:])
                    lb = psm.tile([128, OWN], F32, tag="lb")
                    nc.sync.dma_start(
                        out=lb[:], in_=l_d[h:h + 1, :].to_broadcast((128, OWN))
                    )
                    at_st = psm.tile([128, OWN], F32R, tag="atst")
                    nc.vector.tensor_tensor(at_st[:], psa[:], lb[:], MULT)
                    nc.sync.dma_start(
                        out=at_d[h * 128:(h + 1) * 128, :], in_=at_st[:]
                    )

            # ---------------- phase 4: output projection ----------------
            with (
                tc.tile_pool(name="p4a", bufs=2) as p4a,
                tc.tile_pool(name="p4st", bufs=2) as p4st,
                tc.tile_pool(name="ps_o", bufs=2, space="PSUM") as ps_o,
            ):
                for nh in range(D // ND):
                    won = pslab.tile([128, DK * ND], F32R, tag="slab")
                    nc.sync.dma_start(
                        out=won[:],
                        in_=wo.rearrange("(k p) n -> p k n", p=128)[
                            :, :, nh * ND:(nh + 1) * ND
                        ],
                    )
                    for tt in range(OWN // 128):
                        a_t = p4a.tile([128, DK * 128], F32R, tag="at")
                        nc.sync.dma_start(
                            out=a_t[:],
                            in_=at_d.rearrange("(k p) t -> p k t", p=128)[
                                :, :, tt * 128:(tt + 1) * 128
                            ],
                        )
                        pso = ps_o.tile([128, ND], F32, tag="pso")
                        for k in range(DK):
                            for sc in range(ND // min(512, ND)):
                                NO = min(512, ND)
                                nc.tensor.matmul(
                                    pso[:, sc * NO:(sc + 1) * NO],
                                    _r(a_t[:, k * 128:(k + 1) * 128]),
                                    _r(won[:, k * ND + sc * NO:
                                           k * ND + (sc + 1) * NO]),
                                    start=(k == 0),
                                    stop=False,
                                )
                        for sc in range(ND // min(512, ND)):
                            NO = min(512, ND)
                            nc.tensor.matmul(
                                pso[:, sc * NO:(sc + 1) * NO],
                                _r(ones_row[:]),
                                _r(bo_sb[:, nh * ND + sc * NO:
                                         nh * ND + (sc + 1) * NO]),
                                start=False,
                                stop=True,
                            )
                        ost = p4st.tile([128, ND], F32, tag="ost")
                        nc.scalar.copy(ost[:], pso[:])
                        nc.sync.dma_start(
                            out=o[tt * 128:(tt + 1) * 128, nh * ND:(nh + 1) * ND],
                            in_=ost[:],
                        )
    nc.compile()
    return nc


def host_shard(cfg, x_full):
    """Per-core permutations, permuted x, and mask tensors.

    Returns (perms, x_ins, masks): lists indexed by core = 2*b + z.
    """
    c = _derived(cfg)
    B, T, QW, OWN, q = c["B"], c["T"], c["QW"], c["OWN"], c["q"]
    quarters = [np.arange(i * QW, (i + 1) * QW) for i in range(4)]
    LWIN = list(range(q)) + list(range(2 * q, 3 * q))
    HONLY = list(range(q, 2 * q)) + list(range(3 * q, 4 * q))
    perms, x_ins, masks = [], [], []
    for b in range(B):
        for z in range(2):
            if z == 0:
                own = [quarters[0], quarters[3]]
                rest = [quarters[1], quarters[2]]
            else:
                own = [quarters[1], quarters[2]]
                rest = [quarters[0], quarters[3]]
            perm = np.concatenate(own + rest)
            perms.append(perm)
            x_ins.append(np.ascontiguousarray(x_full[b][perm]))
            m = np.empty((128, 4 * q * QW), dtype=np.float32)
            ig_L = perm[:QW]
            ig_H = perm[QW:OWN]
            for t, jb in enumerate(LWIN):
                jg = perm[jb * 128:(jb + 1) * 128]
                m[:, t * QW:(t + 1) * QW] = (
                    jg[:, None] <= ig_L[None, :]
                ).astype(np.float32)
            for t, jb in enumerate(HONLY):
                jg = perm[jb * 128:(jb + 1) * 128]
                m[:, (2 * q + t) * QW:(2 * q + t + 1) * QW] = (
                    jg[:, None] <= ig_H[None, :]
                ).astype(np.float32)
            masks.append(m)
    return perms, x_ins, masks


def run_cores(cfg, nc, inputs, perms, x_ins, masks, trace=False, tmpdir=None):
    c = _derived(cfg)
    n = c["N_CORES"]
    f32 = np.float32
    shared = {
        "wq": np.ascontiguousarray(inputs["Wq"], f32),
        "wk": np.ascontiguousarray(inputs["Wk"], f32),
        "wv": np.ascontiguousarray(inputs["Wv"], f32),
        "wo": np.ascontiguousarray(inputs["Wo"], f32),
        "bq": np.ascontiguousarray(inputs["bq"], f32),
        "bk": np.ascontiguousarray(inputs["bk"], f32),
        "bv": np.ascontiguousarray(inputs["bv"], f32),
        "bo": np.ascontiguousarray(inputs["bo"], f32),
    }
    consts = {
        "ident": np.eye(128, dtype=f32),
        "ones_c": np.ones((128, 1), f32),
        "ones_r": np.ones((1, 128), f32),
    }
    in_maps = [
        {"x": x_ins[i], "mask": masks[i], **consts, **shared} for i in range(n)
    ]
    res = run_bass_kernel_spmd(
        nc, in_maps, list(range(n)), trace=trace, tmpdir=tmpdir
    )
    B, T, D, OWN = c["B"], c["T"], c["D"], c["OWN"]
    out = np.empty((B, T, D), dtype=np.float32)
    for b in range(B):
        for z in range(2):
            core = 2 * b + z
            out[b][perms[core][:OWN]] = res.results[core]["o"]
    return out, res


_NC_CACHE = {}


def kernel(x, Wq, bq, Wk, bk, Wv, bv, Wo, bo):
    cfg = PROD_CFG
    key = tuple(sorted(cfg.items()))
    if key not in _NC_CACHE:
        _NC_CACHE[key] = build_nc(cfg)
    nc = _NC_CACHE[key]
    x = np.asarray(x, np.float32)
    perms, x_ins, masks = host_shard(cfg, x)
    inputs = dict(Wq=Wq, bq=bq, Wk=Wk, bk=bk, Wv=Wv, bv=bv, Wo=Wo, bo=bo)
    out, _ = run_cores(cfg, nc, inputs, perms, x_ins, masks)
    return out


# revision 9
# speedup vs baseline: 1.0719x; 1.0719x over previous
"""Causal multi-head attention (B=4, T=2048, D=2048, H=16) on 8 Trainium2
NeuronCores via Bass/Tile, SPMD with zero collectives.

Sharding: each batch b is split over two cores by query rows using a
zigzag quarter split (core 2b: quarters Q1+Q4, core 2b+1: quarters Q2+Q3),
which balances the causal-attention triangle. Every core computes K/V
projections for its batch's full sequence (cheap redundancy that keeps the
SPMD program uniform across cores); causal masking is supplied as per-core
input data over a uniform tile pattern, so all 8 cores run the same
instruction stream.

Per-core pipeline (all matmuls in float32r — full PE rate, ~1e-4 rel err):
  0. PE-transpose x -> xT (SBUF slabs, one T/2 half at a time)
  1. K^T = Wk^T xT, Q^T = Wq^T xT (transposed layouts), V = x Wv (natural)
  2. per head: S^T tiles = K^T_chunk^T Q^T, exp on ACT (no max subtraction:
     scores are O(1) by construction), causal/pad masking by DVE multiply,
     A^T accumulated on PE with V as stationary operand, softmax denominators
     via ones-vector matmuls, normalization fused into the PSUM evacuation
  3. O rows = A^T^T Wo + bo
Outputs are the core's own (permuted) query rows; the host scatters them
back into the full [B, T, D] tensor.
"""
import numpy as np

import concourse.bacc as bacc
import concourse.mybir as mybir
from concourse.tile import TileContext
from concourse.bass_utils import run_bass_kernel_spmd

F32 = mybir.dt.float32
F32R = mybir.dt.float32r
EXP = mybir.ActivationFunctionType.Exp
MULT = mybir.AluOpType.mult

PROD_CFG = dict(B=4, T=2048, D=2048, H=16)


def _derived(cfg):
    B, T, D, H = cfg["B"], cfg["T"], cfg["D"], cfg["H"]
    d = dict(cfg)
    d.update(
        QW=T // 4,            # quarter width (query-row shard unit)
        OWN=T // 2,           # own query rows per core
        T2=T // 2,            # xT slab half width
        DK=D // 128,          # contraction chunks
        q=T // 4 // 128,      # 128-row j-tiles per quarter
        NCH=min(512, T // 2),  # moving-N chunk for projections
        ND=min(1024, D),      # phase-4 output-column half
        DH=128,
        N_CORES=2 * B,
    )
    return d


def _r(ap):
    return ap.bitcast(F32R)


def build_nc(cfg):
    c = _derived(cfg)
    T, D, H = c["T"], c["D"], c["H"]
    QW, OWN, T2, DK, q = c["QW"], c["OWN"], c["T2"], c["DK"], c["q"]
    NCH, ND = c["NCH"], c["ND"]
    SCALE = float(c["DH"] ** -0.5)

    nc = bacc.Bacc(
        "TRN2", target_bir_lowering=False, debug=False, num_devices=c["N_CORES"]
    )
    x = nc.dram_tensor("x", [T, D], F32R, kind="ExternalInput").ap()
    wq = nc.dram_tensor("wq", [D, D], F32R, kind="ExternalInput").ap()
    wk = nc.dram_tensor("wk", [D, D], F32R, kind="ExternalInput").ap()
    wv = nc.dram_tensor("wv", [D, D], F32R, kind="ExternalInput").ap()
    wo = nc.dram_tensor("wo", [D, D], F32R, kind="ExternalInput").ap()
    bq = nc.dram_tensor("bq", [D], F32, kind="ExternalInput").ap()
    bk = nc.dram_tensor("bk", [D], F32, kind="ExternalInput").ap()
    bv = nc.dram_tensor("bv", [D], F32, kind="ExternalInput").ap()
    bo = nc.dram_tensor("bo", [D], F32, kind="ExternalInput").ap()
    mask = nc.dram_tensor("mask", [128, 4 * q * QW], F32R, kind="ExternalInput").ap()
    ident_in = nc.dram_tensor("ident", [128, 128], F32R, kind="ExternalInput").ap()
    ones_c_in = nc.dram_tensor("ones_c", [128, 1], F32R, kind="ExternalInput").ap()
    ones_r_in = nc.dram_tensor("ones_r", [1, 128], F32R, kind="ExternalInput").ap()
    o = nc.dram_tensor("o", [OWN, D], F32, kind="ExternalOutput").ap()

    kt_d = nc.dram_tensor("kt_scratch", [D, T], F32R).ap()
    qt_d = nc.dram_tensor("qt_scratch", [D, OWN], F32R).ap()
    v_d = nc.dram_tensor("v_scratch", [T, D], F32R).ap()
    at_d = nc.dram_tensor("at_scratch", [D, OWN], F32R).ap()
    l_d = nc.dram_tensor("l_scratch", [H, OWN], F32).ap()

    # uniform causal j-tile windows (see module docstring)
    LWIN = list(range(q)) + list(range(2 * q, 3 * q))          # L+H valid
    HONLY = list(range(q, 2 * q)) + list(range(3 * q, 4 * q))  # H valid only

    with TileContext(nc) as tc:
        with (
            tc.tile_pool(name="const", bufs=1) as pconst,
        ):
            ident = pconst.tile([128, 128], F32R, tag="ident")
            nc.sync.dma_start(out=ident[:], in_=ident_in[:])
            ones_col = pconst.tile([128, 1], F32R, tag="ones_col")
            nc.sync.dma_start(out=ones_col[:], in_=ones_c_in[:])
            ones_row = pconst.tile([1, 128], F32R, tag="ones_row")
            nc.sync.dma_start(out=ones_row[:], in_=ones_r_in[:])
            bk_sb = pconst.tile([128, DK], F32, tag="bk")
            nc.sync.dma_start(out=bk_sb[:], in_=bk.rearrange("(m p) -> p m", p=128))
            bq_sb = pconst.tile([128, DK], F32, tag="bq")
            nc.sync.dma_start(out=bq_sb[:], in_=bq.rearrange("(m p) -> p m", p=128))
            bv_sb = pconst.tile([1, D], F32R, tag="bv")
            nc.sync.dma_start(out=bv_sb[:], in_=bv[None, :].bitcast(F32R))
            bo_sb = pconst.tile([1, D], F32R, tag="bo")
            nc.sync.dma_start(out=bo_sb[:], in_=bo[None, :].bitcast(F32R))

            # ---------------- phase 0+1: xT, K^T, Q^T, V ----------------
            with (
                tc.tile_pool(name="slab", bufs=1) as pslab,
                tc.tile_pool(name="p1x", bufs=2) as p1x,
                tc.tile_pool(name="p1w", bufs=2) as p1w,
                tc.tile_pool(name="p1wv", bufs=2) as p1wv,
                tc.tile_pool(name="p1st", bufs=3) as p1st,
                tc.tile_pool(name="ps_tr", bufs=2, space="PSUM") as ps_tr,
                tc.tile_pool(name="ps_kq", bufs=2, space="PSUM") as ps_kq,
                tc.tile_pool(name="ps_v", bufs=2, space="PSUM") as ps_v,
            ):
                for hf in range(2):
                    slab = pslab.tile([128, DK * T2], F32R, tag="slab")
                    slab3 = slab[:].rearrange("p (k t) -> p k t", k=DK)
                    # transpose x rows [hf*T2, (hf+1)*T2) into slab
                    for tcn in range(T2 // 128):
                        xst = p1x.tile([128, D], F32R, tag="xst")
                        nc.sync.dma_start(
                            out=xst[:],
                            in_=x[hf * T2 + tcn * 128: hf * T2 + (tcn + 1) * 128, :],
                        )
                        for kb in range(0, DK, 4):
                            nb = min(4, DK - kb)
                            ps = ps_tr.tile([128, 512], F32R, tag="pstr")
                            for i in range(nb):
                                nc.tensor.transpose(
                                    ps[:, i * 128:(i + 1) * 128],
                                    xst[:, (kb + i) * 128:(kb + i + 1) * 128],
                                    ident[:],
                                )
                            nc.vector.tensor_copy(
                                slab3[:, kb:kb + nb, tcn * 128:(tcn + 1) * 128],
                                ps[:, : nb * 128].rearrange(
                                    "p (a b) -> p a b", a=nb
                                ),
                            )
                    # K^T (and Q^T on half 0) projections
                    projs = [(wk, bk_sb, kt_d, True)]
                    if hf == 0:
                        projs.append((wq, bq_sb, qt_d, False))
                    for w_in, b_sb, out_d, is_k in projs:
                        for m in range(DK):
                            wm = p1w.tile([128, DK * 128], F32R, tag="wm")
                            nc.sync.dma_start(
                                out=wm[:],
                                in_=w_in.rearrange("(k p) n -> p k n", p=128)[
                                    :, :, m * 128:(m + 1) * 128
                                ],
                            )
                            for jt in range(T2 // NCH):
                                ps = ps_kq.tile([128, NCH], F32, tag="pskq")
                                for k in range(DK):
                                    nc.tensor.matmul(
                                        ps[:],
                                        _r(wm[:, k * 128:(k + 1) * 128]),
                                        _r(slab[:, k * T2 + jt * NCH:
                                                k * T2 + (jt + 1) * NCH]),
                                        start=(k == 0),
                                        stop=(k == DK - 1),
                                    )
                                st = p1st.tile([128, NCH], F32R, tag="kqst")
                                nc.vector.tensor_scalar_add(
                                    st[:], ps[:], b_sb[:, m:m + 1]
                                )
                                col0 = (hf * T2 if is_k else 0) + jt * NCH
                                nc.sync.dma_start(
                                    out=out_d[m * 128:(m + 1) * 128,
                                              col0:col0 + NCH],
                                    in_=st[:],
                                )
                    # V projection (natural layout), n-chunks of 512
                    for nb_ in range(D // min(512, D)):
                        NV = min(512, D)
                        wvn = p1wv.tile([128, DK * NV], F32R, tag="wvn")
                        nc.sync.dma_start(
                            out=wvn[:],
                            in_=wv.rearrange("(k p) n -> p k n", p=128)[
                                :, :, nb_ * NV:(nb_ + 1) * NV
                            ],
                        )
                        for tcn in range(T2 // 128):
                            ps = ps_v.tile([128, NV], F32, tag="psv")
                            for k in range(DK):
                                nc.tensor.matmul(
                                    ps[:],
                                    _r(slab[:, k * T2 + tcn * 128:
                                            k * T2 + (tcn + 1) * 128]),
                                    _r(wvn[:, k * NV:(k + 1) * NV]),
                                    start=(k == 0),
                                    stop=False,
                                )
                            nc.tensor.matmul(
                                ps[:],
                                _r(ones_row[:]),
                                _r(bv_sb[:, nb_ * NV:(nb_ + 1) * NV]),
                                start=False,
                                stop=True,
                            )
                            st = p1st.tile([128, NV], F32R, tag="vst")
                            nc.scalar.copy(st[:], ps[:])
                            nc.sync.dma_start(
                                out=v_d[hf * T2 + tcn * 128:
                                        hf * T2 + (tcn + 1) * 128,
                                        nb_ * NV:(nb_ + 1) * NV],
                                in_=st[:],
                            )

            # ---------------- phase 2+3: attention per head ----------------
            with (
                tc.tile_pool(name="pmask", bufs=1) as pmask,
                tc.tile_pool(name="ph", bufs=2) as ph,
                tc.tile_pool(name="ppt", bufs=3) as ppt,
                tc.tile_pool(name="psm", bufs=2) as psm,
                tc.tile_pool(name="ps_s", bufs=2, space="PSUM") as ps_s,
                tc.tile_pool(name="ps_a", bufs=1, space="PSUM") as ps_a,
                tc.tile_pool(name="ps_l", bufs=1, space="PSUM") as ps_l,
            ):
                mask_sb = pmask.tile([128, 4 * q * QW], F32R, tag="mask")
                nc.sync.dma_start(out=mask_sb[:], in_=mask[:])
                NS = min(512, OWN)
                for h in range(H):
                    kt_h = ph.tile([128, T], F32R, tag="kth")
                    nc.sync.dma_start(
                        out=kt_h[:], in_=kt_d[h * 128:(h + 1) * 128, :]
                    )
                    qt_h = ph.tile([128, OWN], F32R, tag="qth")
                    nc.sync.dma_start(
                        out=qt_h[:], in_=qt_d[h * 128:(h + 1) * 128, :]
                    )
                    v_h = ph.tile([128, T], F32R, tag="vh")
                    nc.sync.dma_start(
                        out=v_h[:].rearrange("p (jb c) -> p jb c", c=128),
                        in_=v_d.rearrange("(jb p) d -> p jb d", p=128)[
                            :, :, h * 128:(h + 1) * 128
                        ],
                    )
                    psa = ps_a.tile([128, OWN], F32, tag="psa")
                    psl = ps_l.tile([1, OWN], F32, tag="psl")
                    order = LWIN + HONLY
                    for idx, jb in enumerate(order):
                        full = idx < len(LWIN)
                        width = OWN if full else QW
                        pss = ps_s.tile([128, OWN], F32, tag="pss")
                        qoff = 0 if full else QW
                        ns = min(NS, width)
                        for sc in range(width // ns):
                            nc.tensor.matmul(
                                pss[:, sc * ns:(sc + 1) * ns],
                                _r(kt_h[:, jb * 128:(jb + 1) * 128]),
                                _r(qt_h[:, qoff + sc * ns: qoff + (sc + 1) * ns]),
                                start=True,
                                stop=True,
                            )
                        pt = ppt.tile([128, OWN], F32R, tag="pt")
                        nc.scalar.activation(
                            pt[:, :width], pss[:, :width], EXP, scale=SCALE
                        )
                        # causal / padding masks (uniform pattern, per-core data)
                        if full:
                            mc = LWIN.index(jb) * QW
                            nc.vector.tensor_mul(
                                pt[:, :QW], pt[:, :QW], mask_sb[:, mc:mc + QW]
                            )
                        else:
                            mc = (2 * q + HONLY.index(jb)) * QW
                            nc.vector.tensor_mul(
                                pt[:, :QW], pt[:, :QW], mask_sb[:, mc:mc + QW]
                            )
                        vt = _r(v_h[:, jb * 128:(jb + 1) * 128])
                        first = idx == 0
                        # start=True clears has_written for the WHOLE bank, so
                        # the H region may only issue a clearing start when it
                        # occupies its own bank; otherwise its first write
                        # relies on overwrite-where-unset after the L clear.
                        h_own_bank = QW * 4 >= 2048
                        first_h = first and h_own_bank
                        last_l = idx == len(LWIN) - 1
                        last_h = idx == len(order) - 1
                        if full:
                            nc.tensor.matmul(
                                psa[:, :QW], vt, _r(pt[:, :QW]),
                                start=first, stop=last_l,
                            )
                            nc.tensor.matmul(
                                psa[:, QW:OWN], vt, _r(pt[:, QW:OWN]),
                                start=first_h, stop=last_h,
                            )
                            nc.tensor.matmul(
                                psl[:, :QW], _r(ones_col[:]), _r(pt[:, :QW]),
                                start=first, stop=last_l,
                            )
                            nc.tensor.matmul(
                                psl[:, QW:OWN], _r(ones_col[:]),
                                _r(pt[:, QW:OWN]),
                                start=first_h, stop=last_h,
                            )
                        else:
                            nc.tensor.matmul(
                                psa[:, QW:OWN], vt, _r(pt[:, :QW]),
                                start=False, stop=last_h,
                            )
                            nc.tensor.matmul(
                                psl[:, QW:OWN], _r(ones_col[:]),
                                _r(pt[:, :QW]),
                                start=False, stop=last_h,
                            )
                    # Evacuate both PSUM accumulators with fast ACT copies so
                    # the next head's matmuls aren't gated on the (slow)
                    # reciprocal / broadcast / normalize chain below.
                    l_raw = psm.tile([1, OWN], F32, tag="lraw")
                    nc.scalar.copy(l_raw[:], psl[:])
                    at_raw = psm.tile([128, OWN], F32, tag="atraw")
                    nc.scalar.copy(at_raw[:], psa[:])
                    l_sb = psm.tile([1, OWN], F32, tag="lsb")
                    nc.vector.reciprocal(l_sb[:], l_raw[:])
                    nc.sync.dma_start(out=l_d[h:h + 1, :], in_=l_sb[:])
                    lb = psm.tile([128, OWN], F32, tag="lb")
                    nc.sync.dma_start(
                        out=lb[:], in_=l_d[h:h + 1, :].to_broadcast((128, OWN))
                    )
                    at_st = psm.tile([128, OWN], F32R, tag="atst")
                    nc.vector.tensor_tensor(at_st[:], at_raw[:], lb[:], MULT)
                    nc.sync.dma_start(
                        out=at_d[h * 128:(h + 1) * 128, :], in_=at_st[:]
                    )

            # ---------------- phase 4: output projection ----------------
            with (
                tc.tile_pool(name="p4w", bufs=2) as p4w,
                tc.tile_pool(name="p4a", bufs=2) as p4a,
                tc.tile_pool(name="p4st", bufs=2) as p4st,
                tc.tile_pool(name="ps_o", bufs=2, space="PSUM") as ps_o,
            ):
                for nh in range(D // ND):
                    won = p4w.tile([128, DK * ND], F32R, tag="won")
                    nc.sync.dma_start(
                        out=won[:],
                        in_=wo.rearrange("(k p) n -> p k n", p=128)[
                            :, :, nh * ND:(nh + 1) * ND
                        ],
                    )
                    for tt in range(OWN // 128):
                        a_t = p4a.tile([128, DK * 128], F32R, tag="at")
                        nc.sync.dma_start(
                            out=a_t[:],
                            in_=at_d.rearrange("(k p) t -> p k t", p=128)[
                                :, :, tt * 128:(tt + 1) * 128
                            ],
                        )
                        pso = ps_o.tile([128, ND], F32, tag="pso")
                        for k in range(DK):
                            for sc in range(ND // min(512, ND)):
                                NO = min(512, ND)
                                nc.tensor.matmul(
                                    pso[:, sc * NO:(sc + 1) * NO],
                                    _r(a_t[:, k * 128:(k + 1) * 128]),
                                    _r(won[:, k * ND + sc * NO:
                                           k * ND + (sc + 1) * NO]),
                                    start=(k == 0),
                                    stop=False,
                                )
                        for sc in range(ND // min(512, ND)):
                            NO = min(512, ND)
                            nc.tensor.matmul(
                                pso[:, sc * NO:(sc + 1) * NO],
                                _r(ones_row[:]),
                                _r(bo_sb[:, nh * ND + sc * NO:
                                         nh * ND + (sc + 1) * NO]),
                                start=False,
                                stop=True,
                            )
                        ost = p4st.tile([128, ND], F32, tag="ost")
                        nc.scalar.copy(ost[:], pso[:])
                        nc.sync.dma_start(
                            out=o[tt * 128:(tt + 1) * 128, nh * ND:(nh + 1) * ND],
                            in_=ost[:],
                        )
    nc.compile()
    return nc


def host_shard(cfg, x_full):
    """Per-core permutations, permuted x, and mask tensors.

    Returns (perms, x_ins, masks): lists indexed by core = 2*b + z.
    """
    c = _derived(cfg)
    B, T, QW, OWN, q = c["B"], c["T"], c["QW"], c["OWN"], c["q"]
    quarters = [np.arange(i * QW, (i + 1) * QW) for i in range(4)]
    LWIN = list(range(q)) + list(range(2 * q, 3 * q))
    HONLY = list(range(q, 2 * q)) + list(range(3 * q, 4 * q))
    perms, x_ins, masks = [], [], []
    for b in range(B):
        for z in range(2):
            if z == 0:
                own = [quarters[0], quarters[3]]
                rest = [quarters[1], quarters[2]]
            else:
                own = [quarters[1], quarters[2]]
                rest = [quarters[0], quarters[3]]
            perm = np.concatenate(own + rest)
            perms.append(perm)
            x_ins.append(np.ascontiguousarray(x_full[b][perm]))
            m = np.empty((128, 4 * q * QW), dtype=np.float32)
            ig_L = perm[:QW]
            ig_H = perm[QW:OWN]
            for t, jb in enumerate(LWIN):
                jg = perm[jb * 128:(jb + 1) * 128]
                m[:, t * QW:(t + 1) * QW] = (
                    jg[:, None] <= ig_L[None, :]
                ).astype(np.float32)
            for t, jb in enumerate(HONLY):
                jg = perm[jb * 128:(jb + 1) * 128]
                m[:, (2 * q + t) * QW:(2 * q + t + 1) * QW] = (
                    jg[:, None] <= ig_H[None, :]
                ).astype(np.float32)
            masks.append(m)
    return perms, x_ins, masks


def run_cores(cfg, nc, inputs, perms, x_ins, masks, trace=False, tmpdir=None):
    c = _derived(cfg)
    n = c["N_CORES"]
    f32 = np.float32
    shared = {
        "wq": np.ascontiguousarray(inputs["Wq"], f32),
        "wk": np.ascontiguousarray(inputs["Wk"], f32),
        "wv": np.ascontiguousarray(inputs["Wv"], f32),
        "wo": np.ascontiguousarray(inputs["Wo"], f32),
        "bq": np.ascontiguousarray(inputs["bq"], f32),
        "bk": np.ascontiguousarray(inputs["bk"], f32),
        "bv": np.ascontiguousarray(inputs["bv"], f32),
        "bo": np.ascontiguousarray(inputs["bo"], f32),
    }
    consts = {
        "ident": np.eye(128, dtype=f32),
        "ones_c": np.ones((128, 1), f32),
        "ones_r": np.ones((1, 128), f32),
    }
    in_maps = [
        {"x": x_ins[i], "mask": masks[i], **consts, **shared} for i in range(n)
    ]
    res = run_bass_kernel_spmd(
        nc, in_maps, list(range(n)), trace=trace, tmpdir=tmpdir
    )
    B, T, D, OWN = c["B"], c["T"], c["D"], c["OWN"]
    out = np.empty((B, T, D), dtype=np.float32)
    for b in range(B):
        for z in range(2):
            core = 2 * b + z
            out[b][perms[core][:OWN]] = res.results[core]["o"]
    return out, res


_NC_CACHE = {}


def kernel(x, Wq, bq, Wk, bk, Wv, bv, Wo, bo):
    cfg = PROD_CFG
    key = tuple(sorted(cfg.items()))
    if key not in _NC_CACHE:
        _NC_CACHE[key] = build_nc(cfg)
    nc = _NC_CACHE[key]
    x = np.asarray(x, np.float32)
    perms, x_ins, masks = host_shard(cfg, x)
    inputs = dict(Wq=Wq, bq=bq, Wk=Wk, bk=bk, Wv=Wv, bv=bv, Wo=Wo, bo=bo)
    out, _ = run_cores(cfg, nc, inputs, perms, x_ins, masks)
    return out
